# revision 9
# baseline (speedup 1.0000x reference)
"""LoRA Linear kernel for 8x TRN2 NeuronCores (Bass/Tile).

Computes  y = x @ W^T + b + 2.0 * ((x @ A^T) @ B^T)   for
  x [4, 2048, 4096] f32, W [4096, 4096], b [4096], A [16, 4096], B [4096, 16].

Strategy:
  - Data-parallel over tokens: 8192 tokens -> 1024 per core.
  - Host-side prep: transpose x and W to contraction-major layout and cast to
    bf16 (fp32 accumulate in PSUM), so the device does zero transposes.
  - LoRA rank-16 path and the bias are folded into the same PSUM accumulation
    as the base matmul: a K=16 matmul against xa^T and a K=1 matmul of
    ones^T @ b.  PSUM is drained via an ACT copy to SBUF, then DMA to DRAM.
  - Each SBUF tile has exactly one producer proc: Ldweights/TensorCopy can
    encode only a single semaphore wait in walrus codegen.
"""

import os

import numpy as np
import ml_dtypes

_BF16 = ml_dtypes.bfloat16

# Problem constants (hardcoded per harness contract).
_B, _S, _D, _O, _R = 4, 2048, 4096, 4096, 16
_T = _B * _S          # 8192 tokens
_NCORES = 8
_TC = _T // _NCORES   # 1024 tokens per core

P = 128
DS = _D // P          # 32 contraction subtiles
NTT = _TC // P        # 8 t-tiles per core
OBW = 512             # o-block width (one PSUM bank of f32)
NOB = _O // OBW       # 8 o-blocks
XA_CH = 512           # token chunk for the xa matmul
NXA = _TC // XA_CH    # 2

_cache = {}

# Set by kernel() when KERNEL_TRACE=1; read by test.py for exec_time_ns.
LAST_RESULT = None


def _build_module():
    import concourse.bass as bass
    import concourse.bacc as bacc
    import concourse.mybir as mybir
    import concourse.tile as tile
    from concourse.bass import ts

    bf16 = mybir.dt.bfloat16
    f32 = mybir.dt.float32

    nc = bacc.Bacc("TRN2", target_bir_lowering=False, debug=False)
    xT_d = nc.dram_tensor("xT", [_D, _TC], bf16, kind="ExternalInput")
    WT_d = nc.dram_tensor("WT", [_D, _O], bf16, kind="ExternalInput")
    AT_d = nc.dram_tensor("AT", [_D, _R], bf16, kind="ExternalInput")
    BT_d = nc.dram_tensor("BT", [_R, _O], bf16, kind="ExternalInput")
    bvec_d = nc.dram_tensor("bvec", [1, _O], bf16, kind="ExternalInput")
    ones_d = nc.dram_tensor("ones", [1, _TC], bf16, kind="ExternalInput")
    out_d = nc.dram_tensor("out", [_TC, _O], f32, kind="ExternalOutput")

    xT_r = xT_d[:, :].rearrange("(ds p) t -> p ds t", p=P)
    WT_r = WT_d[:, :].rearrange("(ds p) o -> p ds o", p=P)
    AT_r = AT_d[:, :].rearrange("(ds p) r -> p ds r", p=P)

    with tile.TileContext(nc) as tc:
        with (
            tc.tile_pool(name="const", bufs=1) as cpool,
            tc.tile_pool(name="wpool", bufs=2) as wpool,
            tc.tile_pool(name="opool", bufs=4) as opool,
            tc.tile_pool(name="ps_mm", bufs=3, space="PSUM") as ps_pool,
            tc.tile_pool(name="ps_xa", bufs=2, space="PSUM") as ps_xa_pool,
        ):
            xT_sb = cpool.tile([P, DS, _TC], bf16)     # 64KB/partition
            AT_sb = cpool.tile([P, DS, _R], bf16)
            BT_sb = cpool.tile([_R, _O], bf16)
            b_sb = cpool.tile([1, _O], bf16)
            ones_sb = cpool.tile([1, _TC], bf16)
            xaT_sb = cpool.tile([_R, _TC], bf16)

            half = _TC // 2
            for h in range(2):
                sl = slice(h * half, (h + 1) * half)
                nc.sync.dma_start(xT_sb[:, :, sl], xT_r[:, :, sl])
            nc.sync.dma_start(AT_sb[:], AT_r[:])
            nc.sync.dma_start(BT_sb[:], BT_d[:, :])
            nc.sync.dma_start(b_sb[:], bvec_d[:, :])
            nc.sync.dma_start(ones_sb[:], ones_d[:, :])

            # xa^T[r, t] = sum_d A^T[d, r] * x^T[d, t], accumulated in PSUM.
            for cx in range(NXA):
                ps_xa = ps_xa_pool.tile([_R, XA_CH], f32)
                for ds in range(DS):
                    nc.tensor.matmul(
                        ps_xa[:],
                        AT_sb[:, ds, :],
                        xT_sb[:, ds, ts(cx, XA_CH)],
                        start=(ds == 0),
                        stop=(ds == DS - 1),
                    )
                nc.vector.tensor_copy(xaT_sb[:, ts(cx, XA_CH)], ps_xa[:])

            for ob in range(NOB):
                WT_blk = wpool.tile([P, DS, OBW], bf16)
                nc.sync.dma_start(WT_blk[:], WT_r[:, :, ts(ob, OBW)])
                for tt in range(NTT):
                    ps = ps_pool.tile([P, OBW], f32)
                    for ds in range(DS):
                        nc.tensor.matmul(
                            ps[:],
                            xT_sb[:, ds, ts(tt, P)],
                            WT_blk[:, ds, :],
                            start=(ds == 0),
                            stop=False,
                        )
                    # LoRA: xa^T.T @ (2 B^T), K=16
                    nc.tensor.matmul(
                        ps[:],
                        xaT_sb[:, ts(tt, P)],
                        BT_sb[:, ts(ob, OBW)],
                        start=False,
                        stop=False,
                    )
                    # bias: ones^T @ b, K=1
                    nc.tensor.matmul(
                        ps[:],
                        ones_sb[:, ts(tt, P)],
                        b_sb[:, ts(ob, OBW)],
                        start=False,
                        stop=True,
                    )
                    ot = opool.tile([P, OBW], f32)
                    nc.scalar.copy(ot[:], ps[:])
                    nc.gpsimd.dma_start(out_d[ts(tt, P), ts(ob, OBW)], ot[:])
    nc.compile()
    return nc


def kernel(x, W, b, lora_A, lora_B):
    global LAST_RESULT
    from concourse.bass_utils import run_bass_kernel_spmd

    if "nc" not in _cache:
        _cache["nc"] = _build_module()
    nc = _cache["nc"]

    xf = np.ascontiguousarray(x.reshape(_T, _D)).astype(_BF16)
    xT = np.ascontiguousarray(xf.T)                              # [D, T]
    WT = np.ascontiguousarray(W.astype(_BF16).T)                 # [D, O]
    AT = np.ascontiguousarray(lora_A.astype(_BF16).T)            # [D, R]
    BT = np.ascontiguousarray((2.0 * lora_B).astype(_BF16).T)    # [R, O]

    in_maps = []
    for c in range(_NCORES):
        in_maps.append(
            {
                "xT": np.ascontiguousarray(xT[:, c * _TC : (c + 1) * _TC]),
                "WT": WT,
                "AT": AT,
                "BT": BT,
                "bvec": b.astype(_BF16)[None, :],
                "ones": np.ones((1, _TC), dtype=_BF16),
            }
        )

    trace = os.environ.get("KERNEL_TRACE", "0") == "1"
    res = run_bass_kernel_spmd(
        nc,
        in_maps,
        core_ids=list(range(_NCORES)),
        trace=trace,
    )
    LAST_RESULT = res

    out = np.concatenate([r["out"] for r in res.results], axis=0)
    return out.reshape(_B, _S, _O).astype(np.float32, copy=False)
